# revision 2
# baseline (speedup 1.0000x reference)
"""Trainium2 Bass kernel for nn_AttentionAggregator (GNN attention aggregation).

8-core SPMD plan:
- Nodes padded to 50176 = 392 windows of 128. Core c owns src/dst rows
  [6272c, 6272(c+1)): computes its shard of new_emb = fb@W + b and
  s_b = new_emb@a_bot, packed as an augmented fp16 table row
  [new_emb(256) | s_b | pad..] of 768B (dma_gather needs 256B-multiple rows).
- AllGather the augmented table (fp16 [50176, 384]).
- Edges are partitioned by src range (sorted input), grouped per 128-src
  window, split by dst half (<25088 vs >=) so gather indices fit int16,
  padded to a uniform T tiles of 128 edge-slots per (window, half).
- Per 1024-slot chunk: dma_gather 768B rows by dst (random, HBM-latency
  bound) + 256B rows from a local replicated s_a table by src (sorted,
  nearly free). Scores exp(elu(sa+sb)) on DVE/ACT.
- Per 128-edge tile: selector matrix eq = (src_local == iota) scaled by
  score; PE matmul sel^T @ [msg | 1] accumulates [128 src, 256+1]
  (aggregate + row_sum) in PSUM per window; evicted/added into an SBUF
  accumulator; finalized as agg / max(row_sum, 0.5).
"""
import math
import numpy as np

import concourse.bass as bass
import concourse.bacc as bacc
import concourse.mybir as mybir
import concourse.tile as tile
from concourse.bass_utils import run_bass_kernel_spmd

# problem constants (hardcoded per harness contract)
NA = 50000
NB = 50000
D = 256
N_CORES = 8
P = 128

NAP = 50176            # padded nodes: 392 windows of 128
R = NAP // N_CORES     # 6272 rows per core
NW = R // P            # 49 windows per core
HALF = NAP // 2        # 25088 (< int16 max)
ES_MAIN = 384          # fp16 elems per augmented table row (768B)
ES_SA = 128            # fp16 elems per s_a replicated row (256B)
CHUNK = 1024           # gather rows per instruction (descriptor-ring limit)

f32 = mybir.dt.float32
f16 = mybir.dt.float16
i16 = mybir.dt.int16


def _wrap_idx16(flat):
    """int16 idx layout for dma_gather: [128, n/16], idx k at [k%16, k//16],
    replicated 8x down partitions."""
    blk = flat.reshape(-1, 16).T  # [16, n/16]
    return np.tile(blk, (8, 1)).astype(np.int16)


def _build_host(feature_a, feature_b, W, b, a, edge_src, edge_dst):
    a = np.asarray(a, np.float32).reshape(2 * D)
    a_top, a_bot = a[:D], a[D:]
    fa = np.zeros((NAP, D), np.float32)
    fa[:NA] = np.asarray(feature_a, np.float32)
    fb = np.zeros((NAP, D), np.float32)
    fb[:NB] = np.asarray(feature_b, np.float32)
    W = np.asarray(W, np.float32)
    b = np.asarray(b, np.float32).reshape(D)

    src = np.asarray(edge_src, np.int64).astype(np.int32)
    dst = np.asarray(edge_dst, np.int64).astype(np.int32)

    # per-core edge ranges (src sorted)
    bounds = np.searchsorted(src, np.arange(N_CORES + 1) * R)
    per_core = []
    TL = TH = 1
    for c in range(N_CORES):
        s = src[bounds[c]:bounds[c + 1]] - c * R
        d = dst[bounds[c]:bounds[c + 1]]
        w = s >> 7
        half = (d >= HALF)
        for S in (0, 1):
            cnt = np.bincount(w[half == bool(S)], minlength=NW)
            t = int(math.ceil(cnt.max() / P)) if len(cnt) else 1
            if S == 0:
                TL = max(TL, t)
            else:
                TH = max(TH, t)
        per_core.append((s, d, w, half))

    def stream_arrays(s, d, w, mask, TS, base):
        slots = NW * TS * P
        ss, dd, ww = s[mask], d[mask], w[mask]
        counts = np.bincount(ww, minlength=NW)
        starts = np.concatenate([[0], np.cumsum(counts)])
        pos = np.arange(len(ss)) - starts[ww]
        slot = ww * TS * P + pos
        idx_flat = np.zeros(slots, np.int16)
        idx_flat[slot] = (dd - base).astype(np.int16)
        sa_flat = np.zeros(slots, np.int16)
        sa_flat[slot] = ss.astype(np.int16)
        sel_flat = np.full(slots, -1.0, np.float16)
        sel_flat[slot] = (ss & 127).astype(np.float16)
        return (_wrap_idx16(idx_flat), _wrap_idx16(sa_flat),
                np.ascontiguousarray(sel_flat.reshape(NW * TS, P).T))

    iota = np.tile(np.arange(P, dtype=np.float16)[None, :], (P, 1))
    in_maps = []
    for c in range(N_CORES):
        s, d, w, half = per_core[c]
        idxL, saL, selL = stream_arrays(s, d, w, ~half, TL, 0)
        idxH, saH, selH = stream_arrays(s, d, w, half, TH, HALF)
        fbT_aug = np.concatenate(
            [fb[c * R:(c + 1) * R].T, np.ones((1, R), np.float32)], axis=0)
        in_maps.append({
            "fbT_aug": np.ascontiguousarray(fbT_aug),
            "faT": np.ascontiguousarray(fa[c * R:(c + 1) * R].T),
            "W_in": W,
            "Wt_in": np.ascontiguousarray(W.T),
            "abot2": np.ascontiguousarray(a_bot.reshape(2, P).T),
            "atop2": np.ascontiguousarray(a_top.reshape(2, P).T),
            "b_row": b.reshape(1, D),
            "abot_row": a_bot.reshape(1, D),
            "iota_mat": iota,
            "idxL": idxL, "idxH": idxH,
            "saidxL": saL, "saidxH": saH,
            "srcselL": selL, "srcselH": selH,
        })
    return in_maps, TL, TH


def _build_program(TL, TH):
    nc = bacc.Bacc("TRN2", num_devices=N_CORES, debug=False, num_swdge_queues=2)

    fbT_aug = nc.dram_tensor("fbT_aug", [D + 1, R], f32, kind="ExternalInput")
    faT = nc.dram_tensor("faT", [D, R], f32, kind="ExternalInput")
    W_in = nc.dram_tensor("W_in", [D, D], f32, kind="ExternalInput")
    Wt_in = nc.dram_tensor("Wt_in", [D, D], f32, kind="ExternalInput")
    abot2 = nc.dram_tensor("abot2", [P, 2], f32, kind="ExternalInput")
    atop2 = nc.dram_tensor("atop2", [P, 2], f32, kind="ExternalInput")
    b_row = nc.dram_tensor("b_row", [1, D], f32, kind="ExternalInput")
    abot_row = nc.dram_tensor("abot_row", [1, D], f32, kind="ExternalInput")
    iota_mat = nc.dram_tensor("iota_mat", [P, P], f16, kind="ExternalInput")
    idx_in = {}
    for nm, TS in (("L", TL), ("H", TH)):
        slots = NW * TS * P
        idx_in["idx" + nm] = nc.dram_tensor("idx" + nm, [P, slots // 16], i16,
                                            kind="ExternalInput")
        idx_in["saidx" + nm] = nc.dram_tensor("saidx" + nm, [P, slots // 16], i16,
                                              kind="ExternalInput")
        idx_in["srcsel" + nm] = nc.dram_tensor("srcsel" + nm, [P, NW * TS], f16,
                                               kind="ExternalInput")
    out = nc.dram_tensor("out", [R, D], f32, kind="ExternalOutput")

    table_shard = nc.dram_tensor("table_shard", [R, ES_MAIN], f16, kind="Internal")
    table_full = nc.dram_tensor("table_full", [NAP, ES_MAIN], f16,
                                kind="Internal", addr_space="Shared")
    sa_rep = nc.dram_tensor("sa_rep", [R, ES_SA], f16, kind="Internal")

    AL = mybir.AluOpType
    ACT = mybir.ActivationFunctionType

    with tile.TileContext(nc) as tc:
        # ---------------- phase A: build aug table shard + s_a table ------
        with tc.tile_pool(name="wp", bufs=1) as wp, \
             tc.tile_pool(name="lhs", bufs=1) as lp, \
             tc.tile_pool(name="apsum", bufs=2, space="PSUM") as app, \
             tc.tile_pool(name="abld", bufs=4) as bp:
            Wt0 = wp.tile([P, D], f32, tag="wt0")
            Wt1 = wp.tile([P, D], f32, tag="wt1")
            nc.sync.dma_start(out=Wt0[:], in_=Wt_in[0:P, :])
            nc.sync.dma_start(out=Wt1[:], in_=Wt_in[P:D, :])
            ab2 = wp.tile([P, 2], f32, tag="ab2")
            at2 = wp.tile([P, 2], f32, tag="at2")
            nc.sync.dma_start(out=ab2[:], in_=abot2[:])
            nc.sync.dma_start(out=at2[:], in_=atop2[:])
            brw = wp.tile([1, D], f32, tag="brw")
            abr = wp.tile([1, D], f32, tag="abr")
            nc.sync.dma_start(out=brw[:], in_=b_row[:])
            nc.sync.dma_start(out=abr[:], in_=abot_row[:])

            # Waug slabs: rows 0..255 = [W | w2], row 256 = [b | b.a_bot]
            Waug0 = wp.tile([P, D + 1], f32, tag="wa0")
            Waug1 = wp.tile([P, D + 1], f32, tag="wa1")
            Waug2 = wp.tile([1, D + 1], f32, tag="wa2")
            nc.sync.dma_start(out=Waug0[:, 0:D], in_=W_in[0:P, :])
            nc.sync.dma_start(out=Waug1[:, 0:D], in_=W_in[P:D, :])
            for m in range(2):
                psw = app.tile([P, 1], f32, tag="psw")
                nc.tensor.matmul(out=psw[:], lhsT=Wt0[:, m * P:(m + 1) * P],
                                 rhs=ab2[:, 0:1], start=True, stop=False)
                nc.tensor.matmul(out=psw[:], lhsT=Wt1[:, m * P:(m + 1) * P],
                                 rhs=ab2[:, 1:2], start=False, stop=True)
                tgt = Waug0 if m == 0 else Waug1
                nc.vector.tensor_copy(out=tgt[:, D:D + 1], in_=psw[:])
            nc.vector.tensor_copy(out=Waug2[0:1, 0:D], in_=brw[:])
            tmp = bp.tile([1, D], f32, tag="tmp")
            nc.vector.tensor_tensor(out=tmp[:], in0=brw[:], in1=abr[:], op=AL.mult)
            nc.vector.tensor_reduce(out=Waug2[0:1, D:D + 1], in_=tmp[:],
                                    axis=mybir.AxisListType.X, op=AL.add)

            fb0 = lp.tile([P, R], f32, tag="fb0")
            fb1 = lp.tile([P, R], f32, tag="fb1")
            fb2 = lp.tile([1, R], f32, tag="fb2")
            nc.sync.dma_start(out=fb0[:], in_=fbT_aug[0:P, :])
            nc.sync.dma_start(out=fb1[:], in_=fbT_aug[P:D, :])
            nc.sync.dma_start(out=fb2[:], in_=fbT_aug[D:D + 1, :])
            fa0 = lp.tile([P, R], f32, tag="fa0")
            fa1 = lp.tile([P, R], f32, tag="fa1")
            nc.sync.dma_start(out=fa0[:], in_=faT[0:P, :])
            nc.sync.dma_start(out=fa1[:], in_=faT[P:D, :])

            for t in range(NW):
                cs = slice(t * P, (t + 1) * P)
                ps = app.tile([P, D + 1], f32, tag="psA")
                nc.tensor.matmul(out=ps[:], lhsT=fb0[:, cs], rhs=Waug0[:],
                                 start=True, stop=False)
                nc.tensor.matmul(out=ps[:], lhsT=fb1[:, cs], rhs=Waug1[:],
                                 start=False, stop=False)
                nc.tensor.matmul(out=ps[:], lhsT=fb2[0:1, cs], rhs=Waug2[:],
                                 start=False, stop=True)
                aug = bp.tile([P, ES_MAIN], f16, tag="aug")
                nc.vector.tensor_copy(out=aug[:, 0:D + 1], in_=ps[:])
                nc.any.memset(aug[:, D + 1:ES_MAIN], 0)
                nc.sync.dma_start(out=table_shard.ap()[cs, :], in_=aug[:])

                ps2 = app.tile([P, 1], f32, tag="psSA")
                nc.tensor.matmul(out=ps2[:], lhsT=fa0[:, cs], rhs=at2[:, 0:1],
                                 start=True, stop=False)
                nc.tensor.matmul(out=ps2[:], lhsT=fa1[:, cs], rhs=at2[:, 1:2],
                                 start=False, stop=True)
                sac = bp.tile([P, 1], f32, tag="sac1")
                nc.vector.tensor_copy(out=sac[:], in_=ps2[:])
                sar = bp.tile([P, ES_SA], f16, tag="sar")
                nc.vector.tensor_copy(out=sar[:],
                                      in_=sac[:].to_broadcast([P, ES_SA]))
                nc.sync.dma_start(out=sa_rep.ap()[cs, :], in_=sar[:])

        nc.gpsimd.collective_compute(
            "AllGather",
            AL.bypass,
            ins=[table_shard.ap()],
            outs=[table_full.ap()],
            replica_groups=[list(range(N_CORES))],
        )

        # ---------------- phase B: gather + aggregate ---------------------
        with tc.tile_pool(name="acc", bufs=1) as accp, \
             tc.tile_pool(name="ix", bufs=1) as ixp, \
             tc.tile_pool(name="g", bufs=6) as gp, \
             tc.tile_pool(name="sag", bufs=6) as sgp, \
             tc.tile_pool(name="sc", bufs=8) as scp, \
             tc.tile_pool(name="eqp", bufs=6) as eqp, \
             tc.tile_pool(name="bpsum", bufs=4, space="PSUM") as pp, \
             tc.tile_pool(name="fin", bufs=4) as fnp:
            acc = accp.tile([P, NW * (D + 1)], f32, tag="acc")
            iota = ixp.tile([P, P], f16, tag="iota")
            nc.sync.dma_start(out=iota[:], in_=iota_mat[:])

            for S, TS, base in (("L", TL, 0), ("H", TH, HALF)):
                slots = NW * TS * P
                n_tiles = NW * TS
                idxs = ixp.tile([P, slots // 16], i16, tag="idx" + S)
                nc.sync.dma_start(out=idxs[:], in_=idx_in["idx" + S].ap())
                saix = ixp.tile([P, slots // 16], i16, tag="saidx" + S)
                nc.sync.dma_start(out=saix[:], in_=idx_in["saidx" + S].ap())
                ssel = ixp.tile([P, n_tiles], f16, tag="srcsel" + S)
                nc.sync.dma_start(out=ssel[:], in_=idx_in["srcsel" + S].ap())

                tbl_ap = table_full.ap() if S == "L" else table_full.ap()[HALF:NAP, :]
                nchunks = (slots + CHUNK - 1) // CHUNK
                cur_ps = None
                for k in range(nchunks):
                    nk = min(CHUNK, slots - k * CHUNK)
                    nt = nk // P
                    gk = gp.tile([P, (CHUNK // P) * ES_MAIN], f16, tag="g")
                    nc.gpsimd.dma_gather(
                        out_ap=gk[:, 0:nt * ES_MAIN].rearrange(
                            "p (n d) -> p n d", d=ES_MAIN),
                        in_ap=tbl_ap,
                        idxs_ap=idxs[:, k * (CHUNK // 16):
                                     k * (CHUNK // 16) + nk // 16],
                        num_idxs=nk, num_idxs_reg=nk, elem_size=ES_MAIN,
                        queue_num=k % 2)
                    sk = sgp.tile([P, (CHUNK // P) * ES_SA], f16, tag="sa")
                    nc.gpsimd.dma_gather(
                        out_ap=sk[:, 0:nt * ES_SA].rearrange(
                            "p (n d) -> p n d", d=ES_SA),
                        in_ap=sa_rep.ap(),
                        idxs_ap=saix[:, k * (CHUNK // 16):
                                     k * (CHUNK // 16) + nk // 16],
                        num_idxs=nk, num_idxs_reg=nk, elem_size=ES_SA,
                        queue_num=(k + 1) % 2)

                    g3 = gk[:, 0:nt * ES_MAIN].rearrange("p (n d) -> p n d",
                                                         d=ES_MAIN)
                    s3 = sk[:, 0:nt * ES_SA].rearrange("p (n d) -> p n d",
                                                       d=ES_SA)
                    lg = scp.tile([P, CHUNK // P], f32, tag="lg")
                    lg3 = lg[:, 0:nt].rearrange("p (n o) -> p n o", o=1)
                    nc.vector.tensor_tensor(out=lg3, in0=s3[:, :, 0:1],
                                            in1=g3[:, :, D:D + 1], op=AL.add)
                    ex = scp.tile([P, CHUNK // P], f32, tag="ex")
                    ex3 = ex[:, 0:nt].rearrange("p (n o) -> p n o", o=1)
                    nc.scalar.activation(out=ex3, in_=lg3, func=ACT.Exp)
                    low = scp.tile([P, CHUNK // P], f32, tag="low")
                    low3 = low[:, 0:nt].rearrange("p (n o) -> p n o", o=1)
                    nc.vector.tensor_scalar(out=low3, in0=ex3, scalar1=0.1,
                                            scalar2=-0.1, op0=AL.mult, op1=AL.add)
                    msk = scp.tile([P, CHUNK // P], f32, tag="msk")
                    msk3 = msk[:, 0:nt].rearrange("p (n o) -> p n o", o=1)
                    nc.vector.tensor_scalar(out=msk3, in0=lg3, scalar1=0.0,
                                            scalar2=None, op0=AL.is_gt)
                    dd = scp.tile([P, CHUNK // P], f32, tag="dd")
                    dd3 = dd[:, 0:nt].rearrange("p (n o) -> p n o", o=1)
                    nc.vector.tensor_tensor(out=dd3, in0=lg3, in1=low3,
                                            op=AL.subtract)
                    nc.vector.tensor_tensor(out=dd3, in0=msk3, in1=dd3,
                                            op=AL.mult)
                    nc.vector.tensor_tensor(out=dd3, in0=low3, in1=dd3,
                                            op=AL.add)
                    scv = scp.tile([P, CHUNK // P], f32, tag="scv")
                    scv3 = scv[:, 0:nt].rearrange("p (n o) -> p n o", o=1)
                    nc.scalar.activation(out=scv3, in_=dd3, func=ACT.Exp)
                    # overwrite the s_b column with ones -> row_sum column
                    nc.any.memset(g3[:, :, D:D + 1], 1.0)

                    for t in range(nt):
                        j = k * (CHUNK // P) + t
                        if j >= n_tiles:
                            break
                        w, tt = divmod(j, TS)
                        eq = eqp.tile([P, P], f16, tag="eq")
                        nc.vector.tensor_tensor(
                            out=eq[:],
                            in0=ssel[:, j:j + 1].to_broadcast([P, P]),
                            in1=iota[:], op=AL.is_equal)
                        sel = eqp.tile([P, P], f16, tag="sel")
                        nc.vector.tensor_scalar(
                            out=sel[:], in0=eq[:], scalar1=scv[:, t:t + 1],
                            scalar2=None, op0=AL.mult)
                        if tt == 0:
                            cur_ps = pp.tile([P, D + 1], f32, tag="pw")
                        nc.tensor.matmul(
                            out=cur_ps[:], lhsT=sel[:],
                            rhs=gk[:, t * ES_MAIN:t * ES_MAIN + D + 1],
                            start=(tt == 0), stop=(tt == TS - 1))
                        if tt == TS - 1:
                            aslice = acc[:, w * (D + 1):(w + 1) * (D + 1)]
                            if S == "L":
                                nc.vector.tensor_copy(out=aslice, in_=cur_ps[:])
                            else:
                                nc.vector.tensor_tensor(out=aslice, in0=aslice,
                                                        in1=cur_ps[:], op=AL.add)

            for w in range(NW):
                abase = w * (D + 1)
                rs = fnp.tile([P, 1], f32, tag="rs")
                nc.vector.tensor_scalar_max(out=rs[:],
                                            in0=acc[:, abase + D:abase + D + 1],
                                            scalar1=0.5)
                inv = fnp.tile([P, 1], f32, tag="inv")
                nc.vector.reciprocal(out=inv[:], in_=rs[:])
                ot = fnp.tile([P, D], f32, tag="ot")
                nc.vector.tensor_scalar(out=ot[:], in0=acc[:, abase:abase + D],
                                        scalar1=inv[:], scalar2=None,
                                        op0=AL.mult)
                nc.sync.dma_start(out=out.ap()[w * P:(w + 1) * P, :], in_=ot[:])

    nc.compile()
    return nc


def kernel(feature_a, feature_b, W, b, a, edge_src, edge_dst,
           node_num_a=None, node_num_b=None):
    in_maps, TL, TH = _build_host(feature_a, feature_b, W, b, a,
                                  edge_src, edge_dst)
    nc = _build_program(TL, TH)
    res = run_bass_kernel_spmd(nc, in_maps, core_ids=list(range(N_CORES)))
    full = np.concatenate([res.results[c]["out"] for c in range(N_CORES)], axis=0)
    return np.ascontiguousarray(full[:NA]).astype(np.float32)


# revision 3
# speedup vs baseline: 1.4980x; 1.4980x over previous
"""Trainium2 Bass kernel for nn_AttentionAggregator (GNN attention aggregation).

8-core SPMD plan:
- Nodes padded to 50176 = 392 windows of 128. Core c owns src/dst rows
  [6272c, 6272(c+1)): computes its shard of new_emb = fb@W + b and
  s_b = new_emb@a_bot, packed as an augmented fp16 table row
  [new_emb(256) | s_b | pad..] of 768B (dma_gather needs 256B-multiple rows).
- AllGather the augmented table (fp16 [50176, 384]).
- Edges are partitioned by src range (sorted input), grouped per 128-src
  window, split by dst half (<25088 vs >=) so gather indices fit int16,
  padded to a uniform T tiles of 128 edge-slots per (window, half).
- Per 1024-slot chunk: dma_gather 768B rows by dst (random, HBM-latency
  bound) + 256B rows from a local replicated s_a table by src (sorted,
  nearly free). Scores exp(elu(sa+sb)) on DVE/ACT.
- Per 128-edge tile: selector matrix eq = (src_local == iota) scaled by
  score; PE matmul sel^T @ [msg | 1] accumulates [128 src, 256+1]
  (aggregate + row_sum) in PSUM per window; evicted/added into an SBUF
  accumulator; finalized as agg / max(row_sum, 0.5).
"""
import math
import numpy as np

import concourse.bass as bass
import concourse.bacc as bacc
import concourse.mybir as mybir
import concourse.tile as tile
from concourse.bass_utils import run_bass_kernel_spmd

# problem constants (hardcoded per harness contract)
NA = 50000
NB = 50000
D = 256
N_CORES = 8
P = 128

NAP = 50176            # padded nodes: 392 windows of 128
R = NAP // N_CORES     # 6272 rows per core
NW = R // P            # 49 windows per core
HALF = NAP // 2        # 25088 (< int16 max)
ES_MAIN = 384          # fp16 elems per augmented table row (768B)
ES_SA = 128            # fp16 elems per s_a replicated row (256B)
CHUNK = 1024           # gather rows per instruction (descriptor-ring limit)

f32 = mybir.dt.float32
f16 = mybir.dt.float16
i16 = mybir.dt.int16


def _wrap_idx16(flat):
    """int16 idx layout for dma_gather: [128, n/16], idx k at [k%16, k//16],
    replicated 8x down partitions."""
    blk = flat.reshape(-1, 16).T  # [16, n/16]
    return np.tile(blk, (8, 1)).astype(np.int16)


def _build_host(feature_a, feature_b, W, b, a, edge_src, edge_dst):
    a = np.asarray(a, np.float32).reshape(2 * D)
    a_top, a_bot = a[:D], a[D:]
    fa = np.zeros((NAP, D), np.float32)
    fa[:NA] = np.asarray(feature_a, np.float32)
    fb = np.zeros((NAP, D), np.float32)
    fb[:NB] = np.asarray(feature_b, np.float32)
    W = np.asarray(W, np.float32)
    b = np.asarray(b, np.float32).reshape(D)

    src = np.asarray(edge_src, np.int64).astype(np.int32)
    dst = np.asarray(edge_dst, np.int64).astype(np.int32)

    # per-core edge ranges (src sorted)
    bounds = np.searchsorted(src, np.arange(N_CORES + 1) * R)
    per_core = []
    TL = TH = 1
    for c in range(N_CORES):
        s = src[bounds[c]:bounds[c + 1]] - c * R
        d = dst[bounds[c]:bounds[c + 1]]
        w = s >> 7
        half = (d >= HALF)
        for S in (0, 1):
            cnt = np.bincount(w[half == bool(S)], minlength=NW)
            t = int(math.ceil(cnt.max() / P)) if len(cnt) else 1
            if S == 0:
                TL = max(TL, t)
            else:
                TH = max(TH, t)
        per_core.append((s, d, w, half))

    def stream_arrays(s, d, w, mask, TS, base):
        slots = NW * TS * P
        ss, dd, ww = s[mask], d[mask], w[mask]
        counts = np.bincount(ww, minlength=NW)
        starts = np.concatenate([[0], np.cumsum(counts)])
        pos = np.arange(len(ss)) - starts[ww]
        slot = ww * TS * P + pos
        idx_flat = np.zeros(slots, np.int16)
        idx_flat[slot] = (dd - base).astype(np.int16)
        sa_flat = np.zeros(slots, np.int16)
        sa_flat[slot] = ss.astype(np.int16)
        sel_flat = np.full(slots, -1.0, np.float16)
        sel_flat[slot] = (ss & 127).astype(np.float16)
        return (_wrap_idx16(idx_flat), _wrap_idx16(sa_flat),
                np.ascontiguousarray(sel_flat.reshape(NW * TS, P).T))

    iota = np.tile(np.arange(P, dtype=np.float16)[None, :], (P, 1))
    in_maps = []
    for c in range(N_CORES):
        s, d, w, half = per_core[c]
        idxL, saL, selL = stream_arrays(s, d, w, ~half, TL, 0)
        idxH, saH, selH = stream_arrays(s, d, w, half, TH, HALF)
        fbT_aug = np.concatenate(
            [fb[c * R:(c + 1) * R].T, np.ones((1, R), np.float32)], axis=0)
        in_maps.append({
            "fbT_aug": np.ascontiguousarray(fbT_aug),
            "faT": np.ascontiguousarray(fa[c * R:(c + 1) * R].T),
            "W_in": W,
            "Wt_in": np.ascontiguousarray(W.T),
            "abot2": np.ascontiguousarray(a_bot.reshape(2, P).T),
            "atop2": np.ascontiguousarray(a_top.reshape(2, P).T),
            "b_row": b.reshape(1, D),
            "abot_row": a_bot.reshape(1, D),
            "iota_mat": iota,
            "idxL": idxL, "idxH": idxH,
            "saidxL": saL, "saidxH": saH,
            "srcselL": selL, "srcselH": selH,
        })
    return in_maps, TL, TH


def _build_program(TL, TH):
    nc = bacc.Bacc("TRN2", num_devices=N_CORES, debug=False, num_swdge_queues=4)

    fbT_aug = nc.dram_tensor("fbT_aug", [D + 1, R], f32, kind="ExternalInput")
    faT = nc.dram_tensor("faT", [D, R], f32, kind="ExternalInput")
    W_in = nc.dram_tensor("W_in", [D, D], f32, kind="ExternalInput")
    Wt_in = nc.dram_tensor("Wt_in", [D, D], f32, kind="ExternalInput")
    abot2 = nc.dram_tensor("abot2", [P, 2], f32, kind="ExternalInput")
    atop2 = nc.dram_tensor("atop2", [P, 2], f32, kind="ExternalInput")
    b_row = nc.dram_tensor("b_row", [1, D], f32, kind="ExternalInput")
    abot_row = nc.dram_tensor("abot_row", [1, D], f32, kind="ExternalInput")
    iota_mat = nc.dram_tensor("iota_mat", [P, P], f16, kind="ExternalInput")
    idx_in = {}
    for nm, TS in (("L", TL), ("H", TH)):
        slots = NW * TS * P
        idx_in["idx" + nm] = nc.dram_tensor("idx" + nm, [P, slots // 16], i16,
                                            kind="ExternalInput")
        idx_in["saidx" + nm] = nc.dram_tensor("saidx" + nm, [P, slots // 16], i16,
                                              kind="ExternalInput")
        idx_in["srcsel" + nm] = nc.dram_tensor("srcsel" + nm, [P, NW * TS], f16,
                                               kind="ExternalInput")
    out = nc.dram_tensor("out", [R, D], f32, kind="ExternalOutput")

    table_shard = nc.dram_tensor("table_shard", [R, ES_MAIN], f16, kind="Internal")
    table_full = nc.dram_tensor("table_full", [NAP, ES_MAIN], f16,
                                kind="Internal", addr_space="Shared")
    sa_rep = nc.dram_tensor("sa_rep", [R, ES_SA], f16, kind="Internal")

    AL = mybir.AluOpType
    ACT = mybir.ActivationFunctionType

    with tile.TileContext(nc) as tc:
        # ---------------- phase A: build aug table shard + s_a table ------
        with tc.tile_pool(name="wp", bufs=1) as wp, \
             tc.tile_pool(name="lhs", bufs=1) as lp, \
             tc.tile_pool(name="apsum", bufs=2, space="PSUM") as app, \
             tc.tile_pool(name="abld", bufs=4) as bp:
            Wt0 = wp.tile([P, D], f32, tag="wt0")
            Wt1 = wp.tile([P, D], f32, tag="wt1")
            nc.sync.dma_start(out=Wt0[:], in_=Wt_in[0:P, :])
            nc.sync.dma_start(out=Wt1[:], in_=Wt_in[P:D, :])
            ab2 = wp.tile([P, 2], f32, tag="ab2")
            at2 = wp.tile([P, 2], f32, tag="at2")
            nc.sync.dma_start(out=ab2[:], in_=abot2[:])
            nc.sync.dma_start(out=at2[:], in_=atop2[:])
            brw = wp.tile([1, D], f32, tag="brw")
            abr = wp.tile([1, D], f32, tag="abr")
            nc.sync.dma_start(out=brw[:], in_=b_row[:])
            nc.sync.dma_start(out=abr[:], in_=abot_row[:])

            # Waug slabs: rows 0..255 = [W | w2], row 256 = [b | b.a_bot]
            Waug0 = wp.tile([P, D + 1], f32, tag="wa0")
            Waug1 = wp.tile([P, D + 1], f32, tag="wa1")
            Waug2 = wp.tile([1, D + 1], f32, tag="wa2")
            nc.sync.dma_start(out=Waug0[:, 0:D], in_=W_in[0:P, :])
            nc.sync.dma_start(out=Waug1[:, 0:D], in_=W_in[P:D, :])
            for m in range(2):
                psw = app.tile([P, 1], f32, tag="psw")
                nc.tensor.matmul(out=psw[:], lhsT=Wt0[:, m * P:(m + 1) * P],
                                 rhs=ab2[:, 0:1], start=True, stop=False)
                nc.tensor.matmul(out=psw[:], lhsT=Wt1[:, m * P:(m + 1) * P],
                                 rhs=ab2[:, 1:2], start=False, stop=True)
                tgt = Waug0 if m == 0 else Waug1
                nc.vector.tensor_copy(out=tgt[:, D:D + 1], in_=psw[:])
            nc.vector.tensor_copy(out=Waug2[0:1, 0:D], in_=brw[:])
            tmp = bp.tile([1, D], f32, tag="tmp")
            nc.vector.tensor_tensor(out=tmp[:], in0=brw[:], in1=abr[:], op=AL.mult)
            nc.vector.tensor_reduce(out=Waug2[0:1, D:D + 1], in_=tmp[:],
                                    axis=mybir.AxisListType.X, op=AL.add)

            fb0 = lp.tile([P, R], f32, tag="fb0")
            fb1 = lp.tile([P, R], f32, tag="fb1")
            fb2 = lp.tile([1, R], f32, tag="fb2")
            nc.sync.dma_start(out=fb0[:], in_=fbT_aug[0:P, :])
            nc.sync.dma_start(out=fb1[:], in_=fbT_aug[P:D, :])
            nc.sync.dma_start(out=fb2[:], in_=fbT_aug[D:D + 1, :])
            fa0 = lp.tile([P, R], f32, tag="fa0")
            fa1 = lp.tile([P, R], f32, tag="fa1")
            nc.sync.dma_start(out=fa0[:], in_=faT[0:P, :])
            nc.sync.dma_start(out=fa1[:], in_=faT[P:D, :])

            for t in range(NW):
                cs = slice(t * P, (t + 1) * P)
                ps = app.tile([P, D + 1], f32, tag="psA")
                nc.tensor.matmul(out=ps[:], lhsT=fb0[:, cs], rhs=Waug0[:],
                                 start=True, stop=False)
                nc.tensor.matmul(out=ps[:], lhsT=fb1[:, cs], rhs=Waug1[:],
                                 start=False, stop=False)
                nc.tensor.matmul(out=ps[:], lhsT=fb2[0:1, cs], rhs=Waug2[:],
                                 start=False, stop=True)
                aug = bp.tile([P, ES_MAIN], f16, tag="aug")
                nc.vector.tensor_copy(out=aug[:, 0:D + 1], in_=ps[:])
                nc.any.memset(aug[:, D + 1:ES_MAIN], 0)
                nc.sync.dma_start(out=table_shard.ap()[cs, :], in_=aug[:])

                ps2 = app.tile([P, 1], f32, tag="psSA")
                nc.tensor.matmul(out=ps2[:], lhsT=fa0[:, cs], rhs=at2[:, 0:1],
                                 start=True, stop=False)
                nc.tensor.matmul(out=ps2[:], lhsT=fa1[:, cs], rhs=at2[:, 1:2],
                                 start=False, stop=True)
                sac = bp.tile([P, 1], f32, tag="sac1")
                nc.vector.tensor_copy(out=sac[:], in_=ps2[:])
                sar = bp.tile([P, ES_SA], f16, tag="sar")
                nc.vector.tensor_copy(out=sar[:],
                                      in_=sac[:].to_broadcast([P, ES_SA]))
                nc.sync.dma_start(out=sa_rep.ap()[cs, :], in_=sar[:])

        nc.gpsimd.collective_compute(
            "AllGather",
            AL.bypass,
            ins=[table_shard.ap()],
            outs=[table_full.ap()],
            replica_groups=[list(range(N_CORES))],
        )

        # ---------------- phase B: gather + aggregate ---------------------
        with tc.tile_pool(name="acc", bufs=1) as accp, \
             tc.tile_pool(name="ix", bufs=1) as ixp, \
             tc.tile_pool(name="g", bufs=6) as gp, \
             tc.tile_pool(name="sag", bufs=6) as sgp, \
             tc.tile_pool(name="sc", bufs=8) as scp, \
             tc.tile_pool(name="eqp", bufs=6) as eqp, \
             tc.tile_pool(name="bpsum", bufs=4, space="PSUM") as pp, \
             tc.tile_pool(name="fin", bufs=4) as fnp:
            acc = accp.tile([P, NW * (D + 1)], f32, tag="acc")
            iota = ixp.tile([P, P], f16, tag="iota")
            nc.sync.dma_start(out=iota[:], in_=iota_mat[:])

            for S, TS, base in (("L", TL, 0), ("H", TH, HALF)):
                slots = NW * TS * P
                n_tiles = NW * TS
                idxs = ixp.tile([P, slots // 16], i16, tag="idx" + S)
                nc.sync.dma_start(out=idxs[:], in_=idx_in["idx" + S].ap())
                saix = ixp.tile([P, slots // 16], i16, tag="saidx" + S)
                nc.sync.dma_start(out=saix[:], in_=idx_in["saidx" + S].ap())
                ssel = ixp.tile([P, n_tiles], f16, tag="srcsel" + S)
                nc.sync.dma_start(out=ssel[:], in_=idx_in["srcsel" + S].ap())

                tbl_ap = table_full.ap() if S == "L" else table_full.ap()[HALF:NAP, :]
                nchunks = (slots + CHUNK - 1) // CHUNK
                cur_ps = None
                for k in range(nchunks):
                    nk = min(CHUNK, slots - k * CHUNK)
                    nt = nk // P
                    gk = gp.tile([P, (CHUNK // P) * ES_MAIN], f16, tag="g")
                    nc.gpsimd.dma_gather(
                        out_ap=gk[:, 0:nt * ES_MAIN].rearrange(
                            "p (n d) -> p n d", d=ES_MAIN),
                        in_ap=tbl_ap,
                        idxs_ap=idxs[:, k * (CHUNK // 16):
                                     k * (CHUNK // 16) + nk // 16],
                        num_idxs=nk, num_idxs_reg=nk, elem_size=ES_MAIN,
                        queue_num=k % 4)
                    sk = sgp.tile([P, (CHUNK // P) * ES_SA], f16, tag="sa")
                    nc.gpsimd.dma_gather(
                        out_ap=sk[:, 0:nt * ES_SA].rearrange(
                            "p (n d) -> p n d", d=ES_SA),
                        in_ap=sa_rep.ap(),
                        idxs_ap=saix[:, k * (CHUNK // 16):
                                     k * (CHUNK // 16) + nk // 16],
                        num_idxs=nk, num_idxs_reg=nk, elem_size=ES_SA,
                        queue_num=(k + 2) % 4)

                    g3 = gk[:, 0:nt * ES_MAIN].rearrange("p (n d) -> p n d",
                                                         d=ES_MAIN)
                    s3 = sk[:, 0:nt * ES_SA].rearrange("p (n d) -> p n d",
                                                       d=ES_SA)
                    lg = scp.tile([P, CHUNK // P], f32, tag="lg")
                    lg3 = lg[:, 0:nt].rearrange("p (n o) -> p n o", o=1)
                    nc.vector.tensor_tensor(out=lg3, in0=s3[:, :, 0:1],
                                            in1=g3[:, :, D:D + 1], op=AL.add)
                    ex = scp.tile([P, CHUNK // P], f32, tag="ex")
                    ex3 = ex[:, 0:nt].rearrange("p (n o) -> p n o", o=1)
                    nc.scalar.activation(out=ex3, in_=lg3, func=ACT.Exp)
                    low = scp.tile([P, CHUNK // P], f32, tag="low")
                    low3 = low[:, 0:nt].rearrange("p (n o) -> p n o", o=1)
                    nc.vector.tensor_scalar(out=low3, in0=ex3, scalar1=0.1,
                                            scalar2=-0.1, op0=AL.mult, op1=AL.add)
                    msk = scp.tile([P, CHUNK // P], f32, tag="msk")
                    msk3 = msk[:, 0:nt].rearrange("p (n o) -> p n o", o=1)
                    nc.vector.tensor_scalar(out=msk3, in0=lg3, scalar1=0.0,
                                            scalar2=None, op0=AL.is_gt)
                    dd = scp.tile([P, CHUNK // P], f32, tag="dd")
                    dd3 = dd[:, 0:nt].rearrange("p (n o) -> p n o", o=1)
                    nc.vector.tensor_tensor(out=dd3, in0=lg3, in1=low3,
                                            op=AL.subtract)
                    nc.vector.tensor_tensor(out=dd3, in0=msk3, in1=dd3,
                                            op=AL.mult)
                    nc.vector.tensor_tensor(out=dd3, in0=low3, in1=dd3,
                                            op=AL.add)
                    scv = scp.tile([P, CHUNK // P], f32, tag="scv")
                    scv3 = scv[:, 0:nt].rearrange("p (n o) -> p n o", o=1)
                    nc.scalar.activation(out=scv3, in_=dd3, func=ACT.Exp)
                    # overwrite the s_b column with ones -> row_sum column
                    nc.any.memset(g3[:, :, D:D + 1], 1.0)

                    for t in range(nt):
                        j = k * (CHUNK // P) + t
                        if j >= n_tiles:
                            break
                        w, tt = divmod(j, TS)
                        eq = eqp.tile([P, P], f16, tag="eq")
                        nc.vector.tensor_tensor(
                            out=eq[:],
                            in0=ssel[:, j:j + 1].to_broadcast([P, P]),
                            in1=iota[:], op=AL.is_equal)
                        sel = eqp.tile([P, P], f16, tag="sel")
                        nc.vector.tensor_scalar(
                            out=sel[:], in0=eq[:], scalar1=scv[:, t:t + 1],
                            scalar2=None, op0=AL.mult)
                        if tt == 0:
                            cur_ps = pp.tile([P, D + 1], f32, tag="pw")
                        nc.tensor.matmul(
                            out=cur_ps[:], lhsT=sel[:],
                            rhs=gk[:, t * ES_MAIN:t * ES_MAIN + D + 1],
                            start=(tt == 0), stop=(tt == TS - 1))
                        if tt == TS - 1:
                            aslice = acc[:, w * (D + 1):(w + 1) * (D + 1)]
                            if S == "L":
                                nc.vector.tensor_copy(out=aslice, in_=cur_ps[:])
                            else:
                                nc.vector.tensor_tensor(out=aslice, in0=aslice,
                                                        in1=cur_ps[:], op=AL.add)

            for w in range(NW):
                abase = w * (D + 1)
                rs = fnp.tile([P, 1], f32, tag="rs")
                nc.vector.tensor_scalar_max(out=rs[:],
                                            in0=acc[:, abase + D:abase + D + 1],
                                            scalar1=0.5)
                inv = fnp.tile([P, 1], f32, tag="inv")
                nc.vector.reciprocal(out=inv[:], in_=rs[:])
                ot = fnp.tile([P, D], f32, tag="ot")
                nc.vector.tensor_scalar(out=ot[:], in0=acc[:, abase:abase + D],
                                        scalar1=inv[:], scalar2=None,
                                        op0=AL.mult)
                nc.sync.dma_start(out=out.ap()[w * P:(w + 1) * P, :], in_=ot[:])

    nc.compile()
    return nc


def kernel(feature_a, feature_b, W, b, a, edge_src, edge_dst,
           node_num_a=None, node_num_b=None):
    in_maps, TL, TH = _build_host(feature_a, feature_b, W, b, a,
                                  edge_src, edge_dst)
    nc = _build_program(TL, TH)
    res = run_bass_kernel_spmd(nc, in_maps, core_ids=list(range(N_CORES)))
    full = np.concatenate([res.results[c]["out"] for c in range(N_CORES)], axis=0)
    return np.ascontiguousarray(full[:NA]).astype(np.float32)


# revision 4
# speedup vs baseline: 3.5409x; 2.3638x over previous
"""Trainium2 Bass kernel for nn_AttentionAggregator (GNN attention aggregation).

8-core SPMD plan:
- Nodes padded to 50176 = 392 windows of 128. Core c owns src/dst rows
  [6272c, 6272(c+1)): computes its shard of new_emb = fb@W + b and
  s_b = new_emb@a_bot, packed as an augmented fp16 table row
  [new_emb(256) | s_b | pad..] of 768B (dma_gather needs 256B-multiple rows).
- AllGather the augmented table (fp16 [50176, 384]).
- Edges are partitioned by src range (sorted input), grouped per 128-src
  window, split by dst half (<25088 vs >=) so gather indices fit int16,
  padded to a uniform T tiles of 128 edge-slots per (window, half).
- Per 1024-slot chunk: dma_gather 768B rows by dst (random, HBM-latency
  bound) + 256B rows from a local replicated s_a table by src (sorted,
  nearly free). Scores exp(elu(sa+sb)) on DVE/ACT.
- Per 128-edge tile: selector matrix eq = (src_local == iota) scaled by
  score; PE matmul sel^T @ [msg | 1] accumulates [128 src, 256+1]
  (aggregate + row_sum) in PSUM per window; evicted/added into an SBUF
  accumulator; finalized as agg / max(row_sum, 0.5).
"""
import math
import numpy as np

import concourse.bass as bass
import concourse.bacc as bacc
import concourse.mybir as mybir
import concourse.tile as tile
from concourse.bass_utils import run_bass_kernel_spmd

# problem constants (hardcoded per harness contract)
NA = 50000
NB = 50000
D = 256
N_CORES = 8
P = 128

NAP = 50176            # padded nodes: 392 windows of 128
R = NAP // N_CORES     # 6272 rows per core
NW = R // P            # 49 windows per core
HALF = NAP // 2        # 25088 (< int16 max)
NWL = NW // 2          # low-split windows per core (AG1)
NWH = NW - NWL         # high-split windows per core (AG2)
RL = NWL * P           # low shard rows per core
RH = NWH * P
ES_MAIN = 384          # fp16 elems per augmented table row (768B)
ES_SA = 128            # fp16 elems per s_a replicated row (256B)
CHUNK = 1024           # gather rows per instruction (descriptor-ring limit)

f32 = mybir.dt.float32
f16 = mybir.dt.float16
i16 = mybir.dt.int16


def _wrap_idx16(flat):
    """int16 idx layout for dma_gather: [128, n/16], idx k at [k%16, k//16],
    replicated 8x down partitions."""
    blk = flat.reshape(-1, 16).T  # [16, n/16]
    return np.tile(blk, (8, 1)).astype(np.int16)


def _build_host(feature_a, feature_b, W, b, a, edge_src, edge_dst):
    a = np.asarray(a, np.float32).reshape(2 * D)
    a_top, a_bot = a[:D], a[D:]
    fa = np.zeros((NAP, D), np.float32)
    fa[:NA] = np.asarray(feature_a, np.float32)
    fb = np.zeros((NAP, D), np.float32)
    fb[:NB] = np.asarray(feature_b, np.float32)
    W = np.asarray(W, np.float32)
    b = np.asarray(b, np.float32).reshape(D)

    src = np.asarray(edge_src, np.int64).astype(np.int32)
    dst = np.asarray(edge_dst, np.int64).astype(np.int32)

    # per-core edge ranges (src sorted)
    bounds = np.searchsorted(src, np.arange(N_CORES + 1) * R)
    per_core = []
    TL = TH = 1
    for c in range(N_CORES):
        s = src[bounds[c]:bounds[c + 1]] - c * R
        d = dst[bounds[c]:bounds[c + 1]]
        w = s >> 7
        dc = d // R
        dr = d % R
        half = (dr >= RL)
        row = np.where(half, dc * RH + (dr - RL), dc * RL + dr)
        for S in (0, 1):
            cnt = np.bincount(w[half == bool(S)], minlength=NW)
            t = int(math.ceil(cnt.max() / P)) if len(cnt) else 1
            if S == 0:
                TL = max(TL, t)
            else:
                TH = max(TH, t)
        per_core.append((s, row, w, half))

    def stream_arrays(s, d, w, mask, TS):
        slots = NW * TS * P
        ss, dd, ww = s[mask], d[mask], w[mask]
        counts = np.bincount(ww, minlength=NW)
        starts = np.concatenate([[0], np.cumsum(counts)])
        pos = np.arange(len(ss)) - starts[ww]
        slot = ww * TS * P + pos
        idx_flat = np.zeros(slots, np.int16)
        idx_flat[slot] = dd.astype(np.int16)
        sa_flat = np.zeros(slots, np.int16)
        sa_flat[slot] = ss.astype(np.int16)
        sel_flat = np.full(slots, -1.0, np.float16)
        sel_flat[slot] = (ss & 127).astype(np.float16)
        return (_wrap_idx16(idx_flat), _wrap_idx16(sa_flat),
                np.ascontiguousarray(sel_flat.reshape(NW * TS, P).T))

    iota = np.tile(np.arange(P, dtype=np.float16)[None, :], (P, 1))
    in_maps = []
    for c in range(N_CORES):
        s, d, w, half = per_core[c]
        idxL, saL, selL = stream_arrays(s, d, w, ~half, TL)
        idxH, saH, selH = stream_arrays(s, d, w, half, TH)
        fbT_aug = np.concatenate(
            [fb[c * R:(c + 1) * R].T, np.ones((1, R), np.float32)], axis=0)
        in_maps.append({
            "fbT_aug": np.ascontiguousarray(fbT_aug),
            "faT": np.ascontiguousarray(fa[c * R:(c + 1) * R].T),
            "W_in": W,
            "Wt_in": np.ascontiguousarray(W.T),
            "abot2": np.ascontiguousarray(a_bot.reshape(2, P).T),
            "atop2": np.ascontiguousarray(a_top.reshape(2, P).T),
            "b_row": b.reshape(1, D),
            "abot_row": a_bot.reshape(1, D),
            "iota_mat": iota,
            "idxL": idxL, "idxH": idxH,
            "saidxL": saL, "saidxH": saH,
            "srcselL": selL, "srcselH": selH,
        })
    return in_maps, TL, TH


def _build_program(TL, TH):
    nc = bacc.Bacc("TRN2", num_devices=N_CORES, debug=False, num_swdge_queues=4)

    fbT_aug = nc.dram_tensor("fbT_aug", [D + 1, R], f32, kind="ExternalInput")
    faT = nc.dram_tensor("faT", [D, R], f32, kind="ExternalInput")
    W_in = nc.dram_tensor("W_in", [D, D], f32, kind="ExternalInput")
    Wt_in = nc.dram_tensor("Wt_in", [D, D], f32, kind="ExternalInput")
    abot2 = nc.dram_tensor("abot2", [P, 2], f32, kind="ExternalInput")
    atop2 = nc.dram_tensor("atop2", [P, 2], f32, kind="ExternalInput")
    b_row = nc.dram_tensor("b_row", [1, D], f32, kind="ExternalInput")
    abot_row = nc.dram_tensor("abot_row", [1, D], f32, kind="ExternalInput")
    iota_mat = nc.dram_tensor("iota_mat", [P, P], f16, kind="ExternalInput")
    idx_in = {}
    for nm, TS in (("L", TL), ("H", TH)):
        slots = NW * TS * P
        idx_in["idx" + nm] = nc.dram_tensor("idx" + nm, [P, slots // 16], i16,
                                            kind="ExternalInput")
        idx_in["saidx" + nm] = nc.dram_tensor("saidx" + nm, [P, slots // 16], i16,
                                              kind="ExternalInput")
        idx_in["srcsel" + nm] = nc.dram_tensor("srcsel" + nm, [P, NW * TS], f16,
                                               kind="ExternalInput")
    out = nc.dram_tensor("out", [R, D], f32, kind="ExternalOutput")

    table_shardL = nc.dram_tensor("table_shardL", [RL, ES_MAIN], f16, kind="Internal")
    table_shardH = nc.dram_tensor("table_shardH", [RH, ES_MAIN], f16, kind="Internal")
    table_fullL = nc.dram_tensor("table_fullL", [N_CORES * RL, ES_MAIN], f16,
                                 kind="Internal", addr_space="Shared")
    table_fullH = nc.dram_tensor("table_fullH", [N_CORES * RH, ES_MAIN], f16,
                                 kind="Internal", addr_space="Shared")
    sa_rep = nc.dram_tensor("sa_rep", [R, ES_SA], f16, kind="Internal")

    AL = mybir.AluOpType
    ACT = mybir.ActivationFunctionType

    with tile.TileContext(nc) as tc:
        # ---------------- phase A: build aug table shard + s_a table ------
        with tc.tile_pool(name="wp", bufs=1) as wp, \
             tc.tile_pool(name="lhs", bufs=1) as lp, \
             tc.tile_pool(name="apsum", bufs=2, space="PSUM") as app, \
             tc.tile_pool(name="abld", bufs=4) as bp:
            Wt0 = wp.tile([P, D], f32, tag="wt0")
            Wt1 = wp.tile([P, D], f32, tag="wt1")
            nc.sync.dma_start(out=Wt0[:], in_=Wt_in[0:P, :])
            nc.sync.dma_start(out=Wt1[:], in_=Wt_in[P:D, :])
            ab2 = wp.tile([P, 2], f32, tag="ab2")
            at2 = wp.tile([P, 2], f32, tag="at2")
            nc.sync.dma_start(out=ab2[:], in_=abot2[:])
            nc.sync.dma_start(out=at2[:], in_=atop2[:])
            brw = wp.tile([1, D], f32, tag="brw")
            abr = wp.tile([1, D], f32, tag="abr")
            nc.sync.dma_start(out=brw[:], in_=b_row[:])
            nc.sync.dma_start(out=abr[:], in_=abot_row[:])

            # Waug slabs: rows 0..255 = [W | w2], row 256 = [b | b.a_bot]
            Waug0 = wp.tile([P, D + 1], f32, tag="wa0")
            Waug1 = wp.tile([P, D + 1], f32, tag="wa1")
            Waug2 = wp.tile([1, D + 1], f32, tag="wa2")
            nc.sync.dma_start(out=Waug0[:, 0:D], in_=W_in[0:P, :])
            nc.sync.dma_start(out=Waug1[:, 0:D], in_=W_in[P:D, :])
            for m in range(2):
                psw = app.tile([P, 1], f32, tag="psw")
                nc.tensor.matmul(out=psw[:], lhsT=Wt0[:, m * P:(m + 1) * P],
                                 rhs=ab2[:, 0:1], start=True, stop=False)
                nc.tensor.matmul(out=psw[:], lhsT=Wt1[:, m * P:(m + 1) * P],
                                 rhs=ab2[:, 1:2], start=False, stop=True)
                tgt = Waug0 if m == 0 else Waug1
                nc.vector.tensor_copy(out=tgt[:, D:D + 1], in_=psw[:])
            nc.vector.tensor_copy(out=Waug2[0:1, 0:D], in_=brw[:])
            tmp = bp.tile([1, D], f32, tag="tmp")
            nc.vector.tensor_tensor(out=tmp[:], in0=brw[:], in1=abr[:], op=AL.mult)
            nc.vector.tensor_reduce(out=Waug2[0:1, D:D + 1], in_=tmp[:],
                                    axis=mybir.AxisListType.X, op=AL.add)

            fb0 = lp.tile([P, R], f32, tag="fb0")
            fb1 = lp.tile([P, R], f32, tag="fb1")
            fb2 = lp.tile([1, R], f32, tag="fb2")
            nc.sync.dma_start(out=fb0[:], in_=fbT_aug[0:P, :])
            nc.sync.dma_start(out=fb1[:], in_=fbT_aug[P:D, :])
            nc.sync.dma_start(out=fb2[:], in_=fbT_aug[D:D + 1, :])
            fa0 = lp.tile([P, R], f32, tag="fa0")
            fa1 = lp.tile([P, R], f32, tag="fa1")
            nc.sync.dma_start(out=fa0[:], in_=faT[0:P, :])
            nc.sync.dma_start(out=fa1[:], in_=faT[P:D, :])

            for t in range(NW):
                cs = slice(t * P, (t + 1) * P)
                ps = app.tile([P, D + 1], f32, tag="psA")
                nc.tensor.matmul(out=ps[:], lhsT=fb0[:, cs], rhs=Waug0[:],
                                 start=True, stop=False)
                nc.tensor.matmul(out=ps[:], lhsT=fb1[:, cs], rhs=Waug1[:],
                                 start=False, stop=False)
                nc.tensor.matmul(out=ps[:], lhsT=fb2[0:1, cs], rhs=Waug2[:],
                                 start=False, stop=True)
                aug = bp.tile([P, ES_MAIN], f16, tag="aug")
                nc.vector.tensor_copy(out=aug[:, 0:D + 1], in_=ps[:])
                nc.any.memset(aug[:, D + 1:ES_MAIN], 0)
                if t < NWL:
                    tcs = slice(t * P, (t + 1) * P)
                    nc.sync.dma_start(out=table_shardL.ap()[tcs, :], in_=aug[:])
                else:
                    tcs = slice((t - NWL) * P, (t - NWL + 1) * P)
                    nc.sync.dma_start(out=table_shardH.ap()[tcs, :], in_=aug[:])

                ps2 = app.tile([P, 1], f32, tag="psSA")
                nc.tensor.matmul(out=ps2[:], lhsT=fa0[:, cs], rhs=at2[:, 0:1],
                                 start=True, stop=False)
                nc.tensor.matmul(out=ps2[:], lhsT=fa1[:, cs], rhs=at2[:, 1:2],
                                 start=False, stop=True)
                sac = bp.tile([P, 1], f32, tag="sac1")
                nc.vector.tensor_copy(out=sac[:], in_=ps2[:])
                sar = bp.tile([P, ES_SA], f16, tag="sar")
                nc.vector.tensor_copy(out=sar[:],
                                      in_=sac[:].to_broadcast([P, ES_SA]))
                nc.sync.dma_start(out=sa_rep.ap()[cs, :], in_=sar[:])

        nc.gpsimd.collective_compute(
            "AllGather",
            AL.bypass,
            ins=[table_shardL.ap()],
            outs=[table_fullL.ap()],
            replica_groups=[list(range(N_CORES))],
        )
        nc.gpsimd.collective_compute(
            "AllGather",
            AL.bypass,
            ins=[table_shardH.ap()],
            outs=[table_fullH.ap()],
            replica_groups=[list(range(N_CORES))],
        )

        # ---------------- phase B: gather + aggregate ---------------------
        with tc.tile_pool(name="acc", bufs=1) as accp, \
             tc.tile_pool(name="ix", bufs=1) as ixp, \
             tc.tile_pool(name="g", bufs=6) as gp, \
             tc.tile_pool(name="sag", bufs=6) as sgp, \
             tc.tile_pool(name="sc", bufs=8) as scp, \
             tc.tile_pool(name="eqp", bufs=6) as eqp, \
             tc.tile_pool(name="bpsum", bufs=4, space="PSUM") as pp, \
             tc.tile_pool(name="fin", bufs=4) as fnp:
            acc = accp.tile([P, NW * (D + 1)], f32, tag="acc")
            iota = ixp.tile([P, P], f16, tag="iota")
            nc.sync.dma_start(out=iota[:], in_=iota_mat[:])

            for S, TS, base in (("L", TL, 0), ("H", TH, HALF)):
                slots = NW * TS * P
                n_tiles = NW * TS
                idxs = ixp.tile([P, slots // 16], i16, tag="idx" + S)
                nc.sync.dma_start(out=idxs[:], in_=idx_in["idx" + S].ap())
                saix = ixp.tile([P, slots // 16], i16, tag="saidx" + S)
                nc.sync.dma_start(out=saix[:], in_=idx_in["saidx" + S].ap())
                ssel = ixp.tile([P, n_tiles], f16, tag="srcsel" + S)
                nc.sync.dma_start(out=ssel[:], in_=idx_in["srcsel" + S].ap())

                tbl_ap = table_fullL.ap() if S == "L" else table_fullH.ap()
                nchunks = (slots + CHUNK - 1) // CHUNK
                cur_ps = None
                for k in range(nchunks):
                    nk = min(CHUNK, slots - k * CHUNK)
                    nt = nk // P
                    gk = gp.tile([P, (CHUNK // P) * ES_MAIN], f16, tag="g")
                    nc.gpsimd.dma_gather(
                        out_ap=gk[:, 0:nt * ES_MAIN].rearrange(
                            "p (n d) -> p n d", d=ES_MAIN),
                        in_ap=tbl_ap,
                        idxs_ap=idxs[:, k * (CHUNK // 16):
                                     k * (CHUNK // 16) + nk // 16],
                        num_idxs=nk, num_idxs_reg=nk, elem_size=ES_MAIN,
                        queue_num=k % 4)
                    sk = sgp.tile([P, (CHUNK // P) * ES_SA], f16, tag="sa")
                    nc.gpsimd.dma_gather(
                        out_ap=sk[:, 0:nt * ES_SA].rearrange(
                            "p (n d) -> p n d", d=ES_SA),
                        in_ap=sa_rep.ap(),
                        idxs_ap=saix[:, k * (CHUNK // 16):
                                     k * (CHUNK // 16) + nk // 16],
                        num_idxs=nk, num_idxs_reg=nk, elem_size=ES_SA,
                        queue_num=(k + 2) % 4)

                    g3 = gk[:, 0:nt * ES_MAIN].rearrange("p (n d) -> p n d",
                                                         d=ES_MAIN)
                    s3 = sk[:, 0:nt * ES_SA].rearrange("p (n d) -> p n d",
                                                       d=ES_SA)
                    lg = scp.tile([P, CHUNK // P], f32, tag="lg")
                    lg3 = lg[:, 0:nt].rearrange("p (n o) -> p n o", o=1)
                    nc.vector.tensor_tensor(out=lg3, in0=s3[:, :, 0:1],
                                            in1=g3[:, :, D:D + 1], op=AL.add)
                    ex = scp.tile([P, CHUNK // P], f32, tag="ex")
                    ex3 = ex[:, 0:nt].rearrange("p (n o) -> p n o", o=1)
                    nc.scalar.activation(out=ex3, in_=lg3, func=ACT.Exp)
                    low = scp.tile([P, CHUNK // P], f32, tag="low")
                    low3 = low[:, 0:nt].rearrange("p (n o) -> p n o", o=1)
                    nc.vector.tensor_scalar(out=low3, in0=ex3, scalar1=0.1,
                                            scalar2=-0.1, op0=AL.mult, op1=AL.add)
                    msk = scp.tile([P, CHUNK // P], f32, tag="msk")
                    msk3 = msk[:, 0:nt].rearrange("p (n o) -> p n o", o=1)
                    nc.vector.tensor_scalar(out=msk3, in0=lg3, scalar1=0.0,
                                            scalar2=None, op0=AL.is_gt)
                    dd = scp.tile([P, CHUNK // P], f32, tag="dd")
                    dd3 = dd[:, 0:nt].rearrange("p (n o) -> p n o", o=1)
                    nc.vector.tensor_tensor(out=dd3, in0=lg3, in1=low3,
                                            op=AL.subtract)
                    nc.vector.tensor_tensor(out=dd3, in0=msk3, in1=dd3,
                                            op=AL.mult)
                    nc.vector.tensor_tensor(out=dd3, in0=low3, in1=dd3,
                                            op=AL.add)
                    scv = scp.tile([P, CHUNK // P], f32, tag="scv")
                    scv3 = scv[:, 0:nt].rearrange("p (n o) -> p n o", o=1)
                    nc.scalar.activation(out=scv3, in_=dd3, func=ACT.Exp)
                    # overwrite the s_b column with ones -> row_sum column
                    nc.any.memset(g3[:, :, D:D + 1], 1.0)

                    for t in range(nt):
                        j = k * (CHUNK // P) + t
                        if j >= n_tiles:
                            break
                        w, tt = divmod(j, TS)
                        eq = eqp.tile([P, P], f16, tag="eq")
                        nc.vector.tensor_tensor(
                            out=eq[:],
                            in0=ssel[:, j:j + 1].to_broadcast([P, P]),
                            in1=iota[:], op=AL.is_equal)
                        sel = eqp.tile([P, P], f16, tag="sel")
                        nc.scalar.activation(out=sel[:], in_=eq[:],
                                             func=ACT.Copy,
                                             scale=scv[:, t:t + 1])
                        if tt == 0:
                            cur_ps = pp.tile([P, D + 1], f32, tag="pw")
                        nc.tensor.matmul(
                            out=cur_ps[:], lhsT=sel[:],
                            rhs=gk[:, t * ES_MAIN:t * ES_MAIN + D + 1],
                            start=(tt == 0), stop=(tt == TS - 1))
                        if tt == TS - 1:
                            aslice = acc[:, w * (D + 1):(w + 1) * (D + 1)]
                            if S == "L":
                                nc.vector.tensor_copy(out=aslice, in_=cur_ps[:])
                            else:
                                nc.vector.tensor_tensor(out=aslice, in0=aslice,
                                                        in1=cur_ps[:], op=AL.add)

            for w in range(NW):
                abase = w * (D + 1)
                rs = fnp.tile([P, 1], f32, tag="rs")
                nc.vector.tensor_scalar_max(out=rs[:],
                                            in0=acc[:, abase + D:abase + D + 1],
                                            scalar1=0.5)
                inv = fnp.tile([P, 1], f32, tag="inv")
                nc.vector.reciprocal(out=inv[:], in_=rs[:])
                ot = fnp.tile([P, D], f32, tag="ot")
                nc.vector.tensor_scalar(out=ot[:], in0=acc[:, abase:abase + D],
                                        scalar1=inv[:], scalar2=None,
                                        op0=AL.mult)
                nc.sync.dma_start(out=out.ap()[w * P:(w + 1) * P, :], in_=ot[:])

    nc.compile()
    return nc


def kernel(feature_a, feature_b, W, b, a, edge_src, edge_dst,
           node_num_a=None, node_num_b=None):
    in_maps, TL, TH = _build_host(feature_a, feature_b, W, b, a,
                                  edge_src, edge_dst)
    nc = _build_program(TL, TH)
    res = run_bass_kernel_spmd(nc, in_maps, core_ids=list(range(N_CORES)))
    full = np.concatenate([res.results[c]["out"] for c in range(N_CORES)], axis=0)
    return np.ascontiguousarray(full[:NA]).astype(np.float32)
